# revision 2
# baseline (speedup 1.0000x reference)
"""Two-layer GAT on 8 Trainium2 NeuronCores.

Strategy (dst-sharded, node-major layout):
 - Nodes are degree-sorted into 128-node blocks; blocks are dealt round-robin
   to the 8 cores so every core runs an identical static schedule (rounds)
   with shared per-round max in-degrees.
 - Per layer, each core builds its shard of a node table
   [row: h(64) | asrc | adst | pad -> 128 f32 = 512B], an AllGather
   replicates the full table, then each round gathers (dma_gather, two
   signed-int16-banked calls) the 512B rows of the in-edge sources of its
   128 nodes, computes t_e = exp(leaky_relu(asrc_src + adst_dst)), and
   accumulates [sum t*h | sum t] in PSUM via identity matmuls.
 - Softmax max-subtraction is algebraically a no-op here (scores are O(10)),
   so exp applies directly; padded slots point at a dummy table row with
   asrc = -1e30 so exp(..) == 0 and they contribute nothing.
"""
import numpy as np

_CACHE = {}


def _host_prep(x, edge_index, cfg):
    N, C, R = cfg["N"], 8, cfg["R"]
    NPC = R * 128            # rows per core shard
    NTOT = C * NPC
    baseA, baseB, span = cfg["baseA"], cfg["baseB"], cfg["span"]
    A_hi = min(NTOT - 1, baseA + span)
    B_lo = max(0, baseB - span)
    assert A_hi >= B_lo - 1

    src = np.asarray(edge_index[0], dtype=np.int64)
    dst = np.asarray(edge_index[1], dtype=np.int64)
    E = src.shape[0]

    deg = np.bincount(dst, minlength=N)
    order = np.argsort(-deg, kind="stable")
    all_nodes = np.concatenate([order, np.full(NTOT - N, -1, dtype=np.int64)])

    m = np.arange(NTOT)
    b = m // 128
    p = m % 128
    rnd = b // C
    core = b % C
    row_of_listpos = NPC * core + 128 * rnd + p

    # bank holes + dummy rows must hold pad nodes
    special_rows = {baseA - 1, baseA, baseB - 1, baseB}
    row_to_listpos = np.empty(NTOT, dtype=np.int64)
    row_to_listpos[row_of_listpos] = m
    pad_positions = [i for i in range(NTOT - 1, -1, -1) if all_nodes[i] < 0]
    pi = 0
    for r in special_rows:
        lp = row_to_listpos[r]
        if all_nodes[lp] >= 0:
            while pi < len(pad_positions):
                q = pad_positions[pi]; pi += 1
                if row_of_listpos[q] not in special_rows and all_nodes[q] < 0:
                    all_nodes[lp], all_nodes[q] = all_nodes[q], all_nodes[lp]
                    break

    node_at_listpos = all_nodes
    row_of_node = np.full(N, -1, dtype=np.int64)
    real = node_at_listpos >= 0
    row_of_node[node_at_listpos[real]] = row_of_listpos[real]

    sr = row_of_node[src]
    dr = row_of_node[dst]

    # bank per edge (0=A, 1=B), balanced per dst node
    canA = sr <= A_hi
    canB = sr >= B_lo
    forcedA = canA & ~canB
    flex = canA & canB
    nA0 = np.bincount(dr[forcedA], minlength=NTOT)
    nf = np.bincount(dr[flex], minlength=NTOT)
    degr = np.bincount(dr, minlength=NTOT)
    cntA = np.clip((degr + 1) // 2, nA0, nA0 + nf)
    o = np.argsort(dr[flex], kind="stable")
    flex_idx = np.nonzero(flex)[0][o]
    grp = dr[flex_idx]
    uniq, first = np.unique(grp, return_index=True)
    fr = np.arange(len(grp)) - first[np.searchsorted(uniq, grp)]
    bank = np.ones(E, dtype=np.int8)
    bank[forcedA] = 0
    bank[flex_idx] = (fr >= (cntA[grp] - nA0[grp])).astype(np.int8)
    cntB = degr - cntA

    rnd_of_node_row = np.empty(NTOT, dtype=np.int64)
    rnd_of_node_row[row_of_listpos] = rnd
    DA = np.zeros(R, dtype=np.int64)
    DB = np.zeros(R, dtype=np.int64)
    np.maximum.at(DA, rnd_of_node_row, cntA)
    np.maximum.at(DB, rnd_of_node_row, cntB)
    DA[(DA + DB) == 0] = 1

    # slot position within (dst, bank); negative gather indices first so the
    # final slot of each call is non-negative (HW drops trailing negatives)
    idxval = np.where(bank == 0, sr - baseA, sr - baseB)
    nonneg = (idxval >= 0).astype(np.int8)
    o2 = np.lexsort((nonneg, bank, dr))
    grp2 = dr[o2] * 2 + bank[o2]
    uniq2, first2 = np.unique(grp2, return_index=True)
    dpos = np.arange(E) - first2[np.searchsorted(uniq2, grp2)]
    d_of_edge = np.empty(E, dtype=np.int64)
    d_of_edge[o2] = dpos

    # guard: if the node at partition 127 of any core fills column dq-1 with a
    # negative idx, the call would end on a trailing negative -> add a pad col
    cnt_nonneg_A = np.bincount(dr[(bank == 0) & (idxval >= 0)], minlength=NTOT)
    cnt_nonneg_B = np.bincount(dr[(bank == 1) & (idxval >= 0)], minlength=NTOT)
    p_of_row = np.empty(NTOT, dtype=np.int64)
    p_of_row[row_of_listpos] = p
    is127 = p_of_row == 127
    fullnegA = is127 & (cntA == DA[rnd_of_node_row]) & (cntA > 0) & (cnt_nonneg_A == 0)
    fullnegB = is127 & (cntB == DB[rnd_of_node_row]) & (cntB > 0) & (cnt_nonneg_B == 0)
    for rr in np.unique(rnd_of_node_row[fullnegA]):
        DA[rr] += 1
    for rr in np.unique(rnd_of_node_row[fullnegB]):
        DB[rr] += 1

    offA = np.concatenate([[0], np.cumsum(DA)])
    offB = np.concatenate([[0], np.cumsum(DB)])
    SA, SB = int(offA[-1]), int(offB[-1])

    idxA = np.zeros((C, SA * 128), dtype=np.int32)
    idxB = np.zeros((C, SB * 128), dtype=np.int32)
    e_core = dr // NPC
    e_rnd = (dr % NPC) // 128
    e_p = dr % 128
    isA = bank == 0
    slotA = (offA[e_rnd[isA]] + d_of_edge[isA]) * 128 + e_p[isA]
    idxA[e_core[isA], slotA] = sr[isA] - baseA
    isB = ~isA
    slotB = (offB[e_rnd[isB]] + d_of_edge[isB]) * 128 + e_p[isB]
    idxB[e_core[isB], slotB] = sr[isB] - baseB
    assert idxA.min() >= -32768 and idxA.max() <= 32766
    assert idxB.min() >= -32768 and idxB.max() <= 32766
    assert not np.any(idxA == -1) and not np.any(idxB == -1)

    def wrap(a):  # [C, S*128] -> [C, 128, S*8] int16 (16-wrap, replicated x8)
        Cn, tot = a.shape
        w = a.reshape(Cn, tot // 16, 16).transpose(0, 2, 1)
        return np.ascontiguousarray(np.tile(w, (1, 8, 1))).astype(np.int16)

    xT = np.zeros((C, x.shape[1], NPC), dtype=np.float32)
    xf = np.asarray(x, dtype=np.float32)
    for k in range(C):
        sel = (core == k) & real
        cols = row_of_listpos[sel] % NPC
        xT[k][:, cols] = xf[node_at_listpos[sel]].T

    return dict(
        idxA=wrap(idxA), idxB=wrap(idxB), xT=xT,
        DA=[int(v) for v in DA], DB=[int(v) for v in DB],
        SA=SA, SB=SB, row_of_node=row_of_node,
    )


def _build(cfg, DA, DB, SA, SB):
    import sys
    if "/opt/trn_rl_repo" not in sys.path:
        sys.path.insert(0, "/opt/trn_rl_repo")
    import concourse.mybir as mybir
    import concourse.tile as tile
    from concourse import bacc
    from concourse.masks import make_identity

    f32 = mybir.dt.float32
    R = cfg["R"]
    F, HD = cfg["F"], cfg["H"]
    NPC = R * 128
    NTOT = 8 * NPC
    baseA, baseB = cfg["baseA"], cfg["baseB"]
    AF = HD + 2  # h | asrc | adst

    nc = bacc.Bacc("TRN2", target_bir_lowering=False, debug=False, num_devices=8)
    xT_t = nc.dram_tensor("xT", [F, NPC], f32, kind="ExternalInput")
    iA_t = nc.dram_tensor("idxA", [128, SA * 8], mybir.dt.int16, kind="ExternalInput")
    iB_t = nc.dram_tensor("idxB", [128, SB * 8], mybir.dt.int16, kind="ExternalInput")
    W1_t = nc.dram_tensor("W1", [F, HD], f32, kind="ExternalInput")
    W2_t = nc.dram_tensor("W2", [HD, HD], f32, kind="ExternalInput")
    av_t = nc.dram_tensor("avec", [4, HD], f32, kind="ExternalInput")
    bv_t = nc.dram_tensor("bvec", [2, HD], f32, kind="ExternalInput")
    out_t = nc.dram_tensor("out", [NPC, HD], f32, kind="ExternalOutput")

    shard1 = nc.dram_tensor("shard1", [NPC, 128], f32, kind="Internal")
    shard2 = nc.dram_tensor("shard2", [NPC, 128], f32, kind="Internal")
    table1 = nc.dram_tensor("table1", [NTOT, 128], f32, kind="Internal", addr_space="Shared")
    table2 = nc.dram_tensor("table2", [NTOT, 128], f32, kind="Internal", addr_space="Shared")

    RG = [[0, 1, 2, 3, 4, 5, 6, 7]]
    DmaxA = max(max(DA), 1)
    DmaxB = max(max(DB), 1)

    with tile.TileContext(nc) as tc:
        with tc.tile_pool(name="const", bufs=1) as cp, \
             tc.tile_pool(name="gpool", bufs=3) as gp, \
             tc.tile_pool(name="mpool", bufs=2) as mp, \
             tc.tile_pool(name="spool", bufs=3) as sp, \
             tc.tile_pool(name="hpool", bufs=max(DA and len(DA), 1) if False else 0 or len(DA)) as hp_pool, \
             tc.tile_pool(name="psA", bufs=2, space="PSUM") as psA, \
             tc.tile_pool(name="psB", bufs=2, space="PSUM") as psB, \
             tc.tile_pool(name="psT", bufs=2, space="PSUM") as psT:

            ident = cp.tile([128, 128], f32)
            make_identity(nc, ident[:])

            # weight prep: aug[l] = [W | W@a_src | W@a_dst]  ([K, AF])
            augs = []
            for l, (Wt, K) in enumerate(((W1_t, F), (W2_t, HD))):
                Wsb = cp.tile([K, HD], f32, tag=f"w{l}")
                nc.sync.dma_start(out=Wsb[:], in_=Wt.ap()[:, :])
                Wt_ps = psT.tile([HD, K], f32, tag="pst")
                nc.tensor.transpose(out=Wt_ps[:], in_=Wsb[:], identity=ident[:K, :K])
                Wtr = cp.tile([HD, K], f32, tag=f"wt{l}")
                nc.vector.tensor_copy(out=Wtr[:], in_=Wt_ps[:])
                aug = cp.tile([K, AF], f32, tag=f"aug{l}")
                nc.vector.tensor_copy(out=aug[:, 0:HD], in_=Wsb[:])
                for s in range(2):
                    acol = cp.tile([HD, 1], f32, tag=f"ac{l}{s}")
                    nc.sync.dma_start(
                        out=acol[:],
                        in_=av_t.ap()[2 * l + s:2 * l + s + 1, :].rearrange("a b -> b a"))
                    wa_ps = psT.tile([K, 1], f32, tag="pst")
                    nc.tensor.matmul(out=wa_ps[:], lhsT=Wtr[:], rhs=acol[:],
                                     start=True, stop=True)
                    nc.vector.tensor_copy(out=aug[:, HD + s:HD + s + 1], in_=wa_ps[:])
                augs.append(aug)

            bb = []
            for l in range(2):
                t = cp.tile([128, HD], f32, tag=f"b{l}")
                nc.sync.dma_start(out=t[:1, :], in_=bv_t.ap()[l:l + 1, :])
                nc.gpsimd.partition_broadcast(t[:], t[:1, :])
                bb.append(t)

            dumrow = cp.tile([1, 128], f32)
            nc.vector.memset(dumrow[:], 0.0)
            nc.vector.memset(dumrow[:, HD:HD + 1], -30.0)

            iA_sb = cp.tile([128, SA * 8], mybir.dt.int16)
            nc.sync.dma_start(out=iA_sb[:], in_=iA_t.ap()[:, :])
            iB_sb = cp.tile([128, SB * 8], mybir.dt.int16)
            nc.sync.dma_start(out=iB_sb[:], in_=iB_t.ap()[:, :])

            offA = [0]
            for d in DA:
                offA.append(offA[-1] + d)
            offB = [0]
            for d in DB:
                offB.append(offB[-1] + d)

            adst_own1 = cp.tile([128, R], f32, tag="adst1")
            adst_own2 = cp.tile([128, R], f32, tag="adst2")
            adst_own = [adst_own1, adst_own2]

            def table_chunk_write(i, hs, shard, layer):
                # hs: SBUF [AF, 128] f-major -> node-major chunk -> shard rows
                htp = psT.tile([128, AF], f32, tag="pst")
                nc.tensor.transpose(out=htp[:], in_=hs[:], identity=ident[:AF, :AF])
                chunk = sp.tile([128, 128], f32, tag="chunk")
                nc.vector.tensor_copy(out=chunk[:, 0:AF], in_=htp[:])
                nc.vector.tensor_copy(out=adst_own[layer][:, i:i + 1],
                                      in_=htp[:, HD + 1:HD + 2])
                nc.sync.dma_start(out=shard.ap()[128 * i:128 * (i + 1), :], in_=chunk[:])

            def phase_A1():
                for t in range(R):
                    rhs = sp.tile([F, 128], f32, tag="parhs")
                    nc.sync.dma_start(out=rhs[:], in_=xT_t.ap()[:, 128 * t:128 * (t + 1)])
                    hp = psA.tile([AF, 128], f32, tag="paps")
                    nc.tensor.matmul(out=hp[:], lhsT=augs[0][:], rhs=rhs[:],
                                     start=True, stop=True)
                    hs = sp.tile([AF, 128], f32, tag="pahs")
                    nc.scalar.copy(out=hs[:], in_=hp[:])
                    table_chunk_write(t, hs, shard1, 0)

            def allgather(shard, table, l):
                nc.gpsimd.collective_compute(
                    "AllGather", mybir.AluOpType.bypass, RG,
                    ins=[shard.ap()[:, :]], outs=[table.ap()[:, :]])
                nc.gpsimd.dma_start(out=table.ap()[baseA:baseA + 1, :], in_=dumrow[:])
                nc.gpsimd.dma_start(out=table.ap()[baseB:baseB + 1, :], in_=dumrow[:])

            def phase_B(layer, table, adst):
                final = layer == 1
                for i in range(R):
                    da, db = DA[i], DB[i]
                    D = da + db
                    GA = gp.tile([128, DmaxA, 128], f32, tag="GA")
                    GB = gp.tile([128, DmaxB, 128], f32, tag="GB")
                    if da:
                        nc.gpsimd.dma_gather(
                            out_ap=GA[:, 0:da, :], in_ap=table.ap()[baseA:, :],
                            idxs_ap=iA_sb[:, offA[i] * 8:offA[i + 1] * 8],
                            num_idxs=128 * da, num_idxs_reg=128 * da,
                            elem_size=128, single_packet=False)
                    if db:
                        nc.gpsimd.dma_gather(
                            out_ap=GB[:, 0:db, :], in_ap=table.ap()[baseB:, :],
                            idxs_ap=iB_sb[:, offB[i] * 8:offB[i + 1] * 8],
                            num_idxs=128 * db, num_idxs_reg=128 * db,
                            elem_size=128, single_packet=False)
                    tb = mp.tile([128, 2], f32, tag="tb")
                    if da:
                        nc.gpsimd.dma_start(out=tb[:, 0:1], in_=GA[:, 0, 0:1])
                        nc.gpsimd.dma_start(out=GA[:, 0, 0:1], in_=tb[:, 0:1])
                    if db:
                        nc.gpsimd.dma_start(out=tb[:, 1:2], in_=GB[:, 0, 0:1])
                        nc.gpsimd.dma_start(out=GB[:, 0, 0:1], in_=tb[:, 1:2])
                    po = psB.tile([128, HD + 1], f32, tag="po")
                    first = True
                    for (G, dq, Dq, btag) in ((GA, da, DmaxA, "a"), (GB, db, DmaxB, "b")):
                        if dq == 0:
                            continue
                        zt = mp.tile([128, Dq, 1], f32, tag="zt" + btag)
                        nc.vector.tensor_scalar(
                            out=zt[:, 0:dq, :], in0=G[:, 0:dq, HD:HD + 1],
                            scalar1=adst[:, i:i + 1], scalar2=None,
                            op0=mybir.AluOpType.add)
                        z2 = mp.tile([128, Dq, 1], f32, tag="z2" + btag)
                        nc.vector.tensor_scalar(
                            out=z2[:, 0:dq, :], in0=zt[:, 0:dq, :],
                            scalar1=cfg["slope"], scalar2=None,
                            op0=mybir.AluOpType.mult)
                        lt = mp.tile([128, Dq, 1], f32, tag="lt" + btag)
                        nc.vector.tensor_tensor(
                            out=lt[:, 0:dq, :], in0=zt[:, 0:dq, :], in1=z2[:, 0:dq, :],
                            op=mybir.AluOpType.max)
                        tt = mp.tile([128, Dq, 1], f32, tag="tt" + btag)
                        nc.scalar.activation(
                            out=tt[:, 0:dq, :], in_=lt[:, 0:dq, :],
                            func=mybir.ActivationFunctionType.Exp)
                        M = mp.tile([128, Dq, HD + 1], f32, tag="M" + btag)
                        nc.vector.tensor_tensor(
                            out=M[:, 0:dq, 0:HD], in0=G[:, 0:dq, 0:HD],
                            in1=tt[:, 0:dq, :].to_broadcast([128, dq, HD]),
                            op=mybir.AluOpType.mult)
                        nc.vector.tensor_copy(out=M[:, 0:dq, HD:HD + 1], in_=tt[:, 0:dq, :])
                        for d in range(dq):
                            nc.tensor.matmul(out=po[:], lhsT=ident[:], rhs=M[:, d, :],
                                             start=first, stop=(btag == "b" or db == 0) and d == dq - 1)
                            first = False
                    den = mp.tile([128, 1], f32, tag="den")
                    nc.vector.tensor_scalar_max(out=den[:], in0=po[:, HD:HD + 1],
                                                scalar1=1e-16)
                    rd = mp.tile([128, 1], f32, tag="rd")
                    nc.vector.reciprocal(out=rd[:], in_=den[:])
                    h = (hp_pool if not final else sp).tile([128, HD], f32, tag="hfin" + str(layer))
                    nc.vector.tensor_scalar_mul(out=h[:], in0=po[:, 0:HD], scalar1=rd[:])
                    nc.vector.tensor_tensor(out=h[:], in0=h[:], in1=bb[layer][:],
                                            op=mybir.AluOpType.add)
                    if final:
                        nc.sync.dma_start(out=out_t.ap()[128 * i:128 * (i + 1), :], in_=h[:])
                    else:
                        nc.scalar.activation(out=h[:], in_=h[:],
                                             func=mybir.ActivationFunctionType.Relu)
                        hkeep.append(h)

            hkeep = []
            phase_A1()
            allgather(shard1, table1, 0)
            phase_B(0, table1, adst_own[0])
            for i in range(R):
                h = hkeep[i]
                htp = psT.tile([HD, 128], f32, tag="pst")
                nc.tensor.transpose(out=htp[:], in_=h[:], identity=ident[:])
                ht = sp.tile([HD, 128], f32, tag="hTs")
                nc.scalar.copy(out=ht[:], in_=htp[:])
                hp2 = psA.tile([AF, 128], f32, tag="paps")
                nc.tensor.matmul(out=hp2[:], lhsT=augs[1][:], rhs=ht[:],
                                 start=True, stop=True)
                hs2 = sp.tile([AF, 128], f32, tag="pahs")
                nc.scalar.copy(out=hs2[:], in_=hp2[:])
                table_chunk_write(i, hs2, shard2, 1)
            allgather(shard2, table2, 1)
            phase_B(1, table2, adst_own[1])

    nc.compile()
    return nc


def _make_cfg(N, F, H):
    if N >= 32768:
        return dict(N=N, R=98, baseA=32768, baseB=67585, span=32766,
                    F=F, H=H, slope=0.2)
    NTOT = max(2048, ((N + 128 + 1023) // 1024) * 1024)
    return dict(N=N, R=NTOT // 1024, baseA=NTOT // 4, baseB=(3 * NTOT) // 4,
                span=min(32766, (5 * NTOT) // 8), F=F, H=H, slope=0.2)


def prepped_run_args(inputs):
    """Build (or fetch cached) nc + per-core input maps for repeat timing."""
    x = np.asarray(inputs["x"])
    cfg = _make_cfg(x.shape[0], x.shape[1], np.asarray(inputs["W1"]).shape[1])
    prep = _host_prep(x, inputs["edge_index"], cfg)
    key = (cfg["N"], cfg["R"], prep["SA"], prep["SB"],
           tuple(prep["DA"]), tuple(prep["DB"]))
    if key not in _CACHE:
        _CACHE[key] = _build(cfg, prep["DA"], prep["DB"], prep["SA"], prep["SB"])
    nc = _CACHE[key]
    avec = np.stack([np.asarray(inputs["a1_src"]), np.asarray(inputs["a1_dst"]),
                     np.asarray(inputs["a2_src"]), np.asarray(inputs["a2_dst"])]).astype(np.float32)
    bvec = np.stack([np.asarray(inputs["b1"]), np.asarray(inputs["b2"])]).astype(np.float32)
    in_maps = []
    for k in range(8):
        in_maps.append({
            "xT": prep["xT"][k], "idxA": prep["idxA"][k], "idxB": prep["idxB"][k],
            "W1": np.asarray(inputs["W1"], dtype=np.float32),
            "W2": np.asarray(inputs["W2"], dtype=np.float32),
            "avec": avec, "bvec": bvec,
        })
    return nc, in_maps


def kernel(x, edge_index, W1, a1_src, a1_dst, b1, W2, a2_src, a2_dst, b2):
    import sys
    if "/opt/trn_rl_repo" not in sys.path:
        sys.path.insert(0, "/opt/trn_rl_repo")
    from concourse import bass_utils

    x = np.asarray(x)
    cfg = _make_cfg(x.shape[0], x.shape[1], np.asarray(W1).shape[1])
    prep = _host_prep(x, edge_index, cfg)
    key = (cfg["N"], cfg["R"], prep["SA"], prep["SB"],
           tuple(prep["DA"]), tuple(prep["DB"]))
    if key not in _CACHE:
        _CACHE[key] = _build(cfg, prep["DA"], prep["DB"], prep["SA"], prep["SB"])
    nc = _CACHE[key]

    avec = np.stack([np.asarray(a1_src), np.asarray(a1_dst),
                     np.asarray(a2_src), np.asarray(a2_dst)]).astype(np.float32)
    bvec = np.stack([np.asarray(b1), np.asarray(b2)]).astype(np.float32)
    in_maps = []
    for k in range(8):
        in_maps.append({
            "xT": prep["xT"][k], "idxA": prep["idxA"][k], "idxB": prep["idxB"][k],
            "W1": np.asarray(W1, dtype=np.float32), "W2": np.asarray(W2, dtype=np.float32),
            "avec": avec, "bvec": bvec,
        })
    res = bass_utils.run_bass_kernel_spmd(nc, in_maps, core_ids=list(range(8)))
    shards = np.concatenate([res.results[k]["out"] for k in range(8)], axis=0)
    return shards[prep["row_of_node"]].astype(np.float32)



# revision 7
# speedup vs baseline: 335.9230x; 335.9230x over previous
"""Two-layer GAT on 8 Trainium2 NeuronCores — replicated-table design.

Strategy (dst-sharded gather, replicated fp16 node tables):
 - Nodes are degree-sorted into 128-node blocks dealt round-robin to the 8
   cores; every core runs an identical static schedule.  Rounds are grouped
   (group-uniform max in-degrees) so per-group ops amortize fixed costs.
 - Layer tables ([h(64) | asrc | pad -> 128 fp16 = 256B rows]) are built by
   EVERY core for ALL nodes from a replicated fp16 x^T input (layer 1) /
   the AllGathered transposed relu(h1) (layer 2).  No table collectives.
 - Per group, two banked dma_gathers (int16 idx range) fetch the 256B rows
   of in-edge sources; scores t_e = exp(leaky_relu(asrc+adst) - 4) computed
   via exp(lrelu(z)) = max(exp(z), exp(0.2 z)) (exp is monotone; the -4
   keeps fp16 products in range; softmax is shift-invariant).  Aggregation
   [sum t*h | sum t] accumulates in PSUM via fp16 identity matmuls.
 - The only collective: AllGather of relu(h1)^T in fp16 (12.8 MB total).
 - Pad gather slots hit dummy rows (asrc = -60) so their t == 0.
"""
import numpy as np

_CACHE = {}

C = 8          # cores
F = 128        # input features
HD = 64        # hidden/output features
AF = HD + 1    # table payload / po columns: h | asrc (or h | den)
ROWW = 128     # table row width (fp16) = 256B
NEG_SLOPE = 0.2
EXP_SHIFT = -4.0
GBUDGET = 64   # max gather slices (A+B) per group
GROUPR = 7     # max rounds per group (PSUM bank: 7*65 <= 512 f32)


def _make_cfg(N):
    R = 98
    NPC = R * 128
    NTOT = C * NPC
    assert NTOT >= N + 4
    return dict(N=N, R=R, NPC=NPC, NTOT=NTOT, baseA=32768, baseB=67585)


def _host_prep(x, edge_index, cfg):
    N, R, NPC, NTOT = cfg["N"], cfg["R"], cfg["NPC"], cfg["NTOT"]
    baseA, baseB = cfg["baseA"], cfg["baseB"]
    A_hi = min(NTOT - 1, baseA + 32766)
    B_lo = baseB - 32768

    src = np.asarray(edge_index[0], dtype=np.int64)
    dst = np.asarray(edge_index[1], dtype=np.int64)
    E = src.shape[0]

    deg = np.bincount(dst, minlength=N)
    order = np.argsort(-deg, kind="stable")
    all_nodes = np.concatenate([order, np.full(NTOT - N, -1, dtype=np.int64)])

    m = np.arange(NTOT)
    b = m // 128
    p = m % 128
    rnd = b // C
    core = b % C
    row_of_listpos = NPC * core + 128 * rnd + p

    # bank holes + dummy rows must hold pad nodes
    special_rows = {baseA - 1, baseA, baseB - 1, baseB}
    row_to_listpos = np.empty(NTOT, dtype=np.int64)
    row_to_listpos[row_of_listpos] = m
    pad_positions = [i for i in range(NTOT - 1, -1, -1) if all_nodes[i] < 0]
    pi = 0
    for r in special_rows:
        lp = row_to_listpos[r]
        if all_nodes[lp] >= 0:
            while pi < len(pad_positions):
                q = pad_positions[pi]; pi += 1
                if row_of_listpos[q] not in special_rows and all_nodes[q] < 0:
                    all_nodes[lp], all_nodes[q] = all_nodes[q], all_nodes[lp]
                    break

    node_at_listpos = all_nodes
    row_of_node = np.full(N, -1, dtype=np.int64)
    real = node_at_listpos >= 0
    row_of_node[node_at_listpos[real]] = row_of_listpos[real]

    sr = row_of_node[src]
    dr = row_of_node[dst]

    # bank per edge (0=A, 1=B), balanced per dst node
    canA = sr <= A_hi
    canB = sr >= B_lo
    forcedA = canA & ~canB
    flex = canA & canB
    nA0 = np.bincount(dr[forcedA], minlength=NTOT)
    nf = np.bincount(dr[flex], minlength=NTOT)
    degr = np.bincount(dr, minlength=NTOT)
    cntA = np.clip((degr + 1) // 2, nA0, nA0 + nf)
    o = np.argsort(dr[flex], kind="stable")
    flex_idx = np.nonzero(flex)[0][o]
    grp = dr[flex_idx]
    uniq, first = np.unique(grp, return_index=True)
    fr = np.arange(len(grp)) - first[np.searchsorted(uniq, grp)]
    bank = np.ones(E, dtype=np.int8)
    bank[forcedA] = 0
    bank[flex_idx] = (fr >= (cntA[grp] - nA0[grp])).astype(np.int8)
    cntB = degr - cntA

    rnd_of_node_row = np.empty(NTOT, dtype=np.int64)
    rnd_of_node_row[row_of_listpos] = rnd
    DA = np.zeros(R, dtype=np.int64)
    DB = np.zeros(R, dtype=np.int64)
    np.maximum.at(DA, rnd_of_node_row, cntA)
    np.maximum.at(DB, rnd_of_node_row, cntB)
    DA[(DA + DB) == 0] = 1

    # greedy grouping: consecutive rounds, group-uniform (da, db), bounded
    # gather-slice budget and PSUM batch width
    groups = []  # [i0, nr, da, db]
    i = 0
    while i < R:
        da, db = int(DA[i]), int(DB[i])
        nr = 1
        while i + nr < R and nr < GROUPR:
            nda = max(da, int(DA[i + nr]))
            ndb = max(db, int(DB[i + nr]))
            if (nr + 1) * (nda + ndb) > GBUDGET:
                break
            da, db = nda, ndb
            nr += 1
        groups.append([i, nr, da, db])
        i += nr

    # slot position within (dst, bank); negative gather indices first so the
    # final slot of each call is non-negative (HW drops trailing negatives)
    idxval = np.where(bank == 0, sr - baseA, sr - baseB)
    nonneg = (idxval >= 0).astype(np.int8)
    o2 = np.lexsort((nonneg, bank, dr))
    grp2 = dr[o2] * 2 + bank[o2]
    uniq2, first2 = np.unique(grp2, return_index=True)
    dpos = np.arange(E) - first2[np.searchsorted(uniq2, grp2)]
    d_of_edge = np.empty(E, dtype=np.int64)
    d_of_edge[o2] = dpos

    # guard: if the node at partition 127 of a group's LAST round fills its
    # final column with a negative idx, the call would end on a trailing
    # negative -> widen the group by a pad col (idx 0 -> dummy row)
    cnt_nonneg_A = np.bincount(dr[(bank == 0) & (idxval >= 0)], minlength=NTOT)
    cnt_nonneg_B = np.bincount(dr[(bank == 1) & (idxval >= 0)], minlength=NTOT)
    p_of_row = np.empty(NTOT, dtype=np.int64)
    p_of_row[row_of_listpos] = p
    is127 = p_of_row == 127
    for g in groups:
        i0, nr, da, db = g
        selr = is127 & (rnd_of_node_row == i0 + nr - 1)
        if np.any(selr & (cntA == da) & (cntA > 0) & (cnt_nonneg_A == 0)):
            g[2] = da + 1
        if np.any(selr & (cntB == db) & (cntB > 0) & (cnt_nonneg_B == 0)):
            g[3] = db + 1

    gbaseA = [0]
    gbaseB = [0]
    for (i0, nr, da, db) in groups:
        gbaseA.append(gbaseA[-1] + nr * da)
        gbaseB.append(gbaseB[-1] + nr * db)
    SA, SB = int(gbaseA[-1]), int(gbaseB[-1])

    rbaseA = np.zeros(R, dtype=np.int64)
    rbaseB = np.zeros(R, dtype=np.int64)
    for gi, (i0, nr, da, db) in enumerate(groups):
        for r in range(nr):
            rbaseA[i0 + r] = gbaseA[gi] + r * da
            rbaseB[i0 + r] = gbaseB[gi] + r * db

    e_core = dr // NPC
    e_rnd = (dr % NPC) // 128
    e_p = dr % 128
    idxA = np.zeros((C, SA * 128), dtype=np.int32)
    idxB = np.zeros((C, SB * 128), dtype=np.int32)
    isA = bank == 0
    slotA = (rbaseA[e_rnd[isA]] + d_of_edge[isA]) * 128 + e_p[isA]
    idxA[e_core[isA], slotA] = sr[isA] - baseA
    isB = ~isA
    slotB = (rbaseB[e_rnd[isB]] + d_of_edge[isB]) * 128 + e_p[isB]
    idxB[e_core[isB], slotB] = sr[isB] - baseB
    assert idxA.min() >= -32768 and idxA.max() <= 32766
    assert idxB.min() >= -32768 and idxB.max() <= 32766
    assert not np.any(idxA == -1) and not np.any(idxB == -1)

    def wrap(a):  # [C, S*128] -> [C, 128, S*8] int16 (16-wrap, replicated x8)
        Cn, tot = a.shape
        w = a.reshape(Cn, tot // 16, 16).transpose(0, 2, 1)
        return np.ascontiguousarray(np.tile(w, (1, 8, 1))).astype(np.int16)

    xf = np.asarray(x, dtype=np.float32)
    xT = np.zeros((F, NTOT), dtype=np.float16)
    xT[:, row_of_listpos[real]] = xf[node_at_listpos[real]].T.astype(np.float16)

    return dict(
        idxA=wrap(idxA), idxB=wrap(idxB), xT=xT,
        groups=[tuple(g) for g in groups], SA=SA, SB=SB,
        row_of_node=row_of_node,
    )


def _build(cfg, groups, SA, SB):
    import sys
    if "/opt/trn_rl_repo" not in sys.path:
        sys.path.insert(0, "/opt/trn_rl_repo")
    import concourse.mybir as mybir
    import concourse.tile as tile
    from concourse import bacc
    from concourse.masks import make_identity

    f32 = mybir.dt.float32
    f16 = mybir.dt.float16
    R, NPC, NTOT = cfg["R"], cfg["NPC"], cfg["NTOT"]
    baseA, baseB = cfg["baseA"], cfg["baseB"]
    NB = NTOT // 128          # 784 chunks
    BCH = 7                   # chunks per build block
    NBLK = NB // BCH          # 112
    RBLK = NPC // 128 // BCH  # 14 own blocks
    GMAXA = max(g[1] * g[2] for g in groups)
    GMAXB = max(g[1] * g[3] for g in groups)

    nc = bacc.Bacc("TRN2", target_bir_lowering=False, debug=False, num_devices=8)
    xT_t = nc.dram_tensor("xT", [F, NTOT], f16, kind="ExternalInput")
    xTo_t = nc.dram_tensor("xTo", [F, NPC], f16, kind="ExternalInput")
    iA_t = nc.dram_tensor("idxA", [128, SA * 8], mybir.dt.int16, kind="ExternalInput")
    iB_t = nc.dram_tensor("idxB", [128, SB * 8], mybir.dt.int16, kind="ExternalInput")
    aug1_t = nc.dram_tensor("aug1", [F, HD + 2], f16, kind="ExternalInput")
    aug2_t = nc.dram_tensor("aug2", [HD, HD + 2], f16, kind="ExternalInput")
    bv_t = nc.dram_tensor("bvec", [2, HD], f32, kind="ExternalInput")
    out_t = nc.dram_tensor("out", [NPC, HD], f32, kind="ExternalOutput")

    table1 = nc.dram_tensor("table1", [NTOT, ROWW], f16, kind="Internal")
    table2 = nc.dram_tensor("table2", [NTOT, ROWW], f16, kind="Internal")
    h1sh = nc.dram_tensor("h1sh", [HD, NPC], f16, kind="Internal")
    h1all = nc.dram_tensor("h1all", [C * HD, NPC], f16, kind="Internal",
                           addr_space="Shared")
    RG = [[0, 1, 2, 3, 4, 5, 6, 7]]

    with tile.TileContext(nc) as tc:
        with tc.tile_pool(name="const", bufs=1) as cp, \
             tc.tile_pool(name="bsrc", bufs=3) as bp, \
             tc.tile_pool(name="bstg", bufs=3) as sp, \
             tc.tile_pool(name="gpool", bufs=2) as gp, \
             tc.tile_pool(name="mpool", bufs=2) as mp, \
             tc.tile_pool(name="psB", bufs=2, space="PSUM") as psB, \
             tc.tile_pool(name="psW", bufs=1, space="PSUM") as psW, \
             tc.tile_pool(name="psT", bufs=2, space="PSUM") as psT:

            ident = cp.tile([128, 128], f16)
            make_identity(nc, ident[:])

            aug1 = cp.tile([F, HD + 2], f16, tag="aug1")
            nc.sync.dma_start(out=aug1[:], in_=aug1_t.ap()[:, :])
            aug2 = cp.tile([HD, HD + 2], f16, tag="aug2")
            nc.sync.dma_start(out=aug2[:], in_=aug2_t.ap()[:, :])
            augs = [aug1, aug2]

            bb = []
            for l in range(2):
                t = cp.tile([128, HD], f32, tag=f"b{l}")
                nc.sync.dma_start(out=t[:1, :], in_=bv_t.ap()[l:l + 1, :])
                nc.gpsimd.partition_broadcast(t[:], t[:1, :])
                bb.append(t)

            shft = cp.tile([128, 1], f32, tag="shft")
            nc.vector.memset(shft[:], EXP_SHIFT)

            dumrow = cp.tile([1, ROWW], f16)
            nc.vector.memset(dumrow[:], 0.0)
            nc.vector.memset(dumrow[:, HD:HD + 1], -60.0)

            iA_sb = cp.tile([128, SA * 8], mybir.dt.int16)
            nc.sync.dma_start(out=iA_sb[:], in_=iA_t.ap()[:, :])
            iB_sb = cp.tile([128, SB * 8], mybir.dt.int16)
            nc.sync.dma_start(out=iB_sb[:], in_=iB_t.ap()[:, :])

            adst0 = cp.tile([128, R], f32, tag="adst0")
            adst1 = cp.tile([128, R], f32, tag="adst1")
            adst = [adst0, adst1]
            h1T = cp.tile([HD, NPC], f16, tag="h1T")

            def build_table(layer, table):
                aug = augs[layer]
                K = F if layer == 0 else HD
                for blk in range(NBLK):
                    if layer == 0:
                        srct = bp.tile([F, BCH * 128], f16, tag="bsrc1")
                        nc.sync.dma_start(
                            out=srct[:],
                            in_=xT_t.ap()[:, BCH * 128 * blk:BCH * 128 * (blk + 1)])
                    else:
                        c0 = BCH * blk
                        rk, r0 = c0 // R, c0 % R
                        srct = bp.tile([HD, BCH * 128], f16, tag="bsrc2")
                        nc.sync.dma_start(
                            out=srct[:],
                            in_=h1all.ap()[HD * rk:HD * (rk + 1),
                                           128 * r0:128 * r0 + BCH * 128])
                    ps = psB.tile([128, BCH * AF], f32, tag="psb")
                    for j in range(BCH):
                        nc.tensor.matmul(
                            out=ps[:, AF * j:AF * (j + 1)],
                            lhsT=srct[:, 128 * j:128 * (j + 1)],
                            rhs=aug[:K, 0:AF], start=True, stop=True)
                    st = sp.tile([128, BCH, AF], f16, tag="bst")
                    nc.scalar.copy(
                        out=st[:],
                        in_=ps[:].rearrange("p (j a) -> p j a", j=BCH))
                    nc.sync.dma_start(
                        out=table.ap()[BCH * 128 * blk:BCH * 128 * (blk + 1),
                                       0:AF].rearrange("(j p) a -> p j a", p=128),
                        in_=st[:])
                nc.gpsimd.dma_start(out=table.ap()[baseA:baseA + 1, :], in_=dumrow[:])
                nc.gpsimd.dma_start(out=table.ap()[baseB:baseB + 1, :], in_=dumrow[:])

            def adst_own_l1():
                for blk in range(RBLK):
                    srct = bp.tile([F, BCH * 128], f16, tag="osrc")
                    nc.sync.dma_start(
                        out=srct[:],
                        in_=xTo_t.ap()[:, BCH * 128 * blk:BCH * 128 * (blk + 1)])
                    ps = psW.tile([128, BCH], f32, tag="pso")
                    for j in range(BCH):
                        nc.tensor.matmul(
                            out=ps[:, j:j + 1],
                            lhsT=srct[:, 128 * j:128 * (j + 1)],
                            rhs=aug1[:, HD + 1:HD + 2], start=True, stop=True)
                    nc.vector.tensor_copy(
                        out=adst[0][:, BCH * blk:BCH * (blk + 1)], in_=ps[:])

            gbA = [0]
            gbB = [0]
            for (i0, nr, da, db) in groups:
                gbA.append(gbA[-1] + nr * da)
                gbB.append(gbB[-1] + nr * db)

            def phase_gather(layer, table):
                final = layer == 1
                for gi, (i0, nr, da, db) in enumerate(groups):
                    GA = gp.tile([128, GMAXA, ROWW], f16, tag="GA")
                    GB = gp.tile([128, GMAXB, ROWW], f16, tag="GB")
                    if da:
                        nc.gpsimd.dma_gather(
                            out_ap=GA[:, 0:nr * da, :], in_ap=table.ap()[baseA:, :],
                            idxs_ap=iA_sb[:, gbA[gi] * 8:gbA[gi + 1] * 8],
                            num_idxs=128 * nr * da, num_idxs_reg=128 * nr * da,
                            elem_size=ROWW, single_packet=False)
                    if db:
                        nc.gpsimd.dma_gather(
                            out_ap=GB[:, 0:nr * db, :], in_ap=table.ap()[baseB:, :],
                            idxs_ap=iB_sb[:, gbB[gi] * 8:gbB[gi + 1] * 8],
                            num_idxs=128 * nr * db, num_idxs_reg=128 * nr * db,
                            elem_size=ROWW, single_packet=False)
                    po = psB.tile([128, GROUPR * AF], f32, tag="po")
                    adcol3 = adst[layer][:, i0:i0 + nr].rearrange(
                        "p (r one) -> p r one", one=1)
                    MM = [None, None]
                    for qi, (G, GMAXQ, dq, tagq) in enumerate(
                            ((GA, GMAXA, da, "a"), (GB, GMAXB, db, "b"))):
                        if dq == 0:
                            continue
                        S = nr * dq
                        asrc3 = G[:, 0:S, HD:HD + 1].rearrange(
                            "p (r d) one -> p r (d one)", r=nr)
                        z = mp.tile([128, GMAXQ], f32, tag="z" + tagq)
                        nc.vector.tensor_tensor(
                            out=z[:, 0:S].rearrange("p (r d) -> p r d", r=nr),
                            in0=asrc3,
                            in1=adcol3.to_broadcast([128, nr, dq]),
                            op=mybir.AluOpType.add)
                        u = mp.tile([128, GMAXQ], f32, tag="u" + tagq)
                        nc.vector.tensor_scalar(
                            out=u[:, 0:S], in0=z[:, 0:S],
                            scalar1=NEG_SLOPE, scalar2=EXP_SHIFT,
                            op0=mybir.AluOpType.mult, op1=mybir.AluOpType.add)
                        e1 = mp.tile([128, GMAXQ], f32, tag="e1" + tagq)
                        nc.scalar.activation(
                            out=e1[:, 0:S], in_=z[:, 0:S],
                            func=mybir.ActivationFunctionType.Exp, bias=shft[:])
                        e2 = mp.tile([128, GMAXQ], f32, tag="e2" + tagq)
                        nc.scalar.activation(
                            out=e2[:, 0:S], in_=u[:, 0:S],
                            func=mybir.ActivationFunctionType.Exp)
                        M = mp.tile([128, GMAXQ, AF], f16, tag="M" + tagq)
                        nc.vector.tensor_tensor(
                            out=M[:, 0:S, HD:HD + 1].rearrange(
                                "p s one -> p (s one)"),
                            in0=e1[:, 0:S], in1=e2[:, 0:S],
                            op=mybir.AluOpType.max)
                        nc.vector.tensor_tensor(
                            out=M[:, 0:S, 0:HD], in0=G[:, 0:S, 0:HD],
                            in1=M[:, 0:S, HD:HD + 1].to_broadcast([128, S, HD]),
                            op=mybir.AluOpType.mult)
                        MM[qi] = M
                    lastq = 1 if MM[1] is not None else 0
                    for r in range(nr):
                        started = False
                        for qi, dq in ((0, da), (1, db)):
                            M = MM[qi]
                            if M is None:
                                continue
                            for d in range(dq):
                                nc.tensor.matmul(
                                    out=po[:, AF * r:AF * (r + 1)],
                                    lhsT=ident[:], rhs=M[:, r * dq + d, :],
                                    start=not started,
                                    stop=(qi == lastq and d == dq - 1))
                                started = True
                    pov = po[:, 0:nr * AF].rearrange("p (r a) -> p r a", r=nr)
                    den = mp.tile([128, GROUPR, 1], f32, tag="den")
                    nc.vector.tensor_scalar_max(
                        out=den[:, 0:nr, :], in0=pov[:, :, HD:HD + 1],
                        scalar1=1e-30)
                    rd = mp.tile([128, GROUPR, 1], f32, tag="rd")
                    nc.vector.reciprocal(out=rd[:, 0:nr, :], in_=den[:, 0:nr, :])
                    h = mp.tile([128, GROUPR, HD], f32, tag="hf")
                    nc.vector.tensor_tensor(
                        out=h[:, 0:nr, :], in0=pov[:, :, 0:HD],
                        in1=rd[:, 0:nr, :].to_broadcast([128, nr, HD]),
                        op=mybir.AluOpType.mult)
                    nc.vector.tensor_tensor(
                        out=h[:, 0:nr, :], in0=h[:, 0:nr, :],
                        in1=bb[layer][:].rearrange(
                            "p (one a) -> p one a", one=1).to_broadcast(
                            [128, nr, HD]),
                        op=mybir.AluOpType.add)
                    if final:
                        nc.sync.dma_start(
                            out=out_t.ap()[128 * i0:128 * (i0 + nr), :].rearrange(
                                "(r p) a -> p r a", p=128),
                            in_=h[:, 0:nr, :])
                    else:
                        hf = mp.tile([128, GROUPR, HD], f16, tag="hfp")
                        nc.scalar.activation(
                            out=hf[:, 0:nr, :], in_=h[:, 0:nr, :],
                            func=mybir.ActivationFunctionType.Relu)
                        pst = psT.tile([HD, GROUPR * 128], f16, tag="pst")
                        for r in range(nr):
                            nc.tensor.transpose(
                                out=pst[:, 128 * r:128 * (r + 1)],
                                in_=hf[:, r, :], identity=ident[:])
                        nc.vector.tensor_copy(
                            out=h1T[:, 128 * i0:128 * (i0 + nr)],
                            in_=pst[:, 0:128 * nr])
                        psd = psW.tile([128, GROUPR], f32, tag="psd")
                        for r in range(nr):
                            nc.tensor.matmul(
                                out=psd[:, r:r + 1],
                                lhsT=h1T[:, 128 * (i0 + r):128 * (i0 + r + 1)],
                                rhs=aug2[:, HD + 1:HD + 2], start=True, stop=True)
                        nc.vector.tensor_copy(
                            out=adst[1][:, i0:i0 + nr], in_=psd[:, 0:nr])

            build_table(0, table1)
            adst_own_l1()
            phase_gather(0, table1)
            nc.sync.dma_start(out=h1sh.ap()[:, :], in_=h1T[:])
            nc.gpsimd.collective_compute(
                "AllGather", mybir.AluOpType.bypass, RG,
                ins=[h1sh.ap()[:, :]], outs=[h1all.ap()[:, :]])
            build_table(1, table2)
            phase_gather(1, table2)

    nc.compile()
    return nc


def _prep_all(inputs):
    x = np.asarray(inputs["x"])
    cfg = _make_cfg(x.shape[0])
    prep = _host_prep(x, inputs["edge_index"], cfg)
    key = (cfg["N"], prep["SA"], prep["SB"], tuple(prep["groups"]))
    if key not in _CACHE:
        _CACHE[key] = _build(cfg, prep["groups"], prep["SA"], prep["SB"])
    nc = _CACHE[key]

    W1 = np.asarray(inputs["W1"], dtype=np.float32)
    W2 = np.asarray(inputs["W2"], dtype=np.float32)
    aug1 = np.concatenate(
        [W1, (W1 @ np.asarray(inputs["a1_src"], np.float32))[:, None],
         (W1 @ np.asarray(inputs["a1_dst"], np.float32))[:, None]], axis=1
    ).astype(np.float16)
    aug2 = np.concatenate(
        [W2, (W2 @ np.asarray(inputs["a2_src"], np.float32))[:, None],
         (W2 @ np.asarray(inputs["a2_dst"], np.float32))[:, None]], axis=1
    ).astype(np.float16)
    bvec = np.stack([np.asarray(inputs["b1"]), np.asarray(inputs["b2"])]
                    ).astype(np.float32)
    NPC = cfg["NPC"]
    in_maps = []
    for k in range(C):
        in_maps.append({
            "xT": prep["xT"],
            "xTo": np.ascontiguousarray(prep["xT"][:, NPC * k:NPC * (k + 1)]),
            "idxA": prep["idxA"][k], "idxB": prep["idxB"][k],
            "aug1": aug1, "aug2": aug2, "bvec": bvec,
        })
    return nc, in_maps, prep, cfg


def prepped_run_args(inputs):
    nc, in_maps, prep, cfg = _prep_all(inputs)
    return nc, in_maps


def kernel(x, edge_index, W1, a1_src, a1_dst, b1, W2, a2_src, a2_dst, b2):
    import sys
    if "/opt/trn_rl_repo" not in sys.path:
        sys.path.insert(0, "/opt/trn_rl_repo")
    from concourse import bass_utils

    inputs = dict(x=x, edge_index=edge_index, W1=W1, a1_src=a1_src,
                  a1_dst=a1_dst, b1=b1, W2=W2, a2_src=a2_src, a2_dst=a2_dst,
                  b2=b2)
    nc, in_maps, prep, cfg = _prep_all(inputs)
    res = bass_utils.run_bass_kernel_spmd(nc, in_maps, core_ids=list(range(C)))
    shards = np.concatenate([res.results[k]["out"] for k in range(C)], axis=0)
    return shards[prep["row_of_node"]].astype(np.float32)


# revision 18
# speedup vs baseline: 423.1750x; 1.2597x over previous
"""Two-layer GAT on 8 Trainium2 NeuronCores — replicated-table design.

Strategy (dst-sharded gather, replicated fp16 node tables):
 - Nodes are degree-sorted into 128-node blocks dealt round-robin to the 8
   cores; every core runs an identical static schedule.  Rounds are grouped
   (group-uniform max in-degrees) so per-group ops amortize fixed costs.
 - Layer tables ([h(64) | asrc | pad -> 128 fp16 = 256B rows]) are built by
   EVERY core for ALL nodes from a replicated fp16 x^T input (layer 1) /
   the AllGathered transposed relu(h1) (layer 2).  No table collectives.
 - Per group, two banked dma_gathers (int16 idx range) fetch the 256B rows
   of in-edge sources; scores t_e = exp(leaky_relu(asrc+adst) - 4) computed
   via exp(lrelu(z)) = max(exp(z), exp(0.2 z)) (exp is monotone; the -4
   keeps fp16 products in range; softmax is shift-invariant).  Aggregation
   [sum t*h | sum t] accumulates in PSUM via fp16 identity matmuls.
 - The only collective: AllGather of relu(h1)^T in fp16 (12.8 MB total).
 - Pad gather slots hit dummy rows (asrc = -60) so their t == 0.
"""
import numpy as np

_CACHE = {}

C = 8          # cores
F = 128        # input features
HD = 64        # hidden/output features
AF = HD + 1    # table payload / po columns: h | asrc (or h | den)
ROWW = 128     # table row width (fp16) = 256B
NEG_SLOPE = 0.2
EXP_SHIFT = -4.0
GBUDGET = 96   # max gather slices (A+B) per group
GROUPR = 7     # max rounds per group (PSUM bank: 7*65 <= 512 f32)
SPLIT_R = 56   # round boundary for the two-piece h1 AllGather (mult of 14)


def _make_cfg(N):
    R = 98
    NPC = R * 128
    NTOT = C * NPC
    assert NTOT >= N + 4
    return dict(N=N, R=R, NPC=NPC, NTOT=NTOT, baseA=32768, baseB=67585)


def _host_prep(x, edge_index, cfg):
    N, R, NPC, NTOT = cfg["N"], cfg["R"], cfg["NPC"], cfg["NTOT"]
    baseA, baseB = cfg["baseA"], cfg["baseB"]
    A_hi = min(NTOT - 1, baseA + 32766)
    B_lo = baseB - 32768

    src = np.asarray(edge_index[0], dtype=np.int64)
    dst = np.asarray(edge_index[1], dtype=np.int64)
    E = src.shape[0]

    deg = np.bincount(dst, minlength=N)
    order = np.argsort(-deg, kind="stable")
    all_nodes = np.concatenate([order, np.full(NTOT - N, -1, dtype=np.int64)])

    m = np.arange(NTOT)
    b = m // 128
    p = m % 128
    rnd = b // C
    core = b % C
    row_of_listpos = NPC * core + 128 * rnd + p

    # bank holes + dummy rows must hold pad nodes
    special_rows = {baseA - 1, baseA, baseB - 1, baseB}
    row_to_listpos = np.empty(NTOT, dtype=np.int64)
    row_to_listpos[row_of_listpos] = m
    pad_positions = [i for i in range(NTOT - 1, -1, -1) if all_nodes[i] < 0]
    pi = 0
    for r in special_rows:
        lp = row_to_listpos[r]
        if all_nodes[lp] >= 0:
            while pi < len(pad_positions):
                q = pad_positions[pi]; pi += 1
                if row_of_listpos[q] not in special_rows and all_nodes[q] < 0:
                    all_nodes[lp], all_nodes[q] = all_nodes[q], all_nodes[lp]
                    break

    node_at_listpos = all_nodes
    row_of_node = np.full(N, -1, dtype=np.int64)
    real = node_at_listpos >= 0
    row_of_node[node_at_listpos[real]] = row_of_listpos[real]

    sr = row_of_node[src]
    dr = row_of_node[dst]

    # bank per edge (0=A, 1=B), balanced per dst node
    canA = sr <= A_hi
    canB = sr >= B_lo
    forcedA = canA & ~canB
    flex = canA & canB
    nA0 = np.bincount(dr[forcedA], minlength=NTOT)
    nf = np.bincount(dr[flex], minlength=NTOT)
    degr = np.bincount(dr, minlength=NTOT)
    # balance each node's A/B split against its ROUND's max degree so the
    # round maxima satisfy maxA+maxB ~ maxdeg (naive per-node halving makes
    # maxA+maxB exceed maxdeg by the binomial spread of the forced edges)
    rnd_of_node_row0 = np.empty(NTOT, dtype=np.int64)
    m0 = np.arange(NTOT)
    rnd_of_node_row0[row_of_listpos] = (m0 // 128) // C
    maxdeg_r = np.zeros(R, dtype=np.int64)
    np.maximum.at(maxdeg_r, rnd_of_node_row0, degr)
    TA = (maxdeg_r + 1) // 2
    TB = maxdeg_r - TA
    TA_row = TA[rnd_of_node_row0]
    TB_row = TB[rnd_of_node_row0]
    lo = np.maximum(nA0, degr - TB_row)
    hi = np.minimum(nA0 + nf, TA_row)
    feasible = lo <= hi
    cntA = np.where(feasible,
                    np.clip((degr + 1) // 2, lo, np.maximum(lo, hi)),
                    np.clip((degr + 1) // 2, nA0, nA0 + nf))
    o = np.argsort(dr[flex], kind="stable")
    flex_idx = np.nonzero(flex)[0][o]
    grp = dr[flex_idx]
    uniq, first = np.unique(grp, return_index=True)
    fr = np.arange(len(grp)) - first[np.searchsorted(uniq, grp)]
    bank = np.ones(E, dtype=np.int8)
    bank[forcedA] = 0
    bank[flex_idx] = (fr >= (cntA[grp] - nA0[grp])).astype(np.int8)
    cntB = degr - cntA

    rnd_of_node_row = np.empty(NTOT, dtype=np.int64)
    rnd_of_node_row[row_of_listpos] = rnd
    DA = np.zeros(R, dtype=np.int64)
    DB = np.zeros(R, dtype=np.int64)
    np.maximum.at(DA, rnd_of_node_row, cntA)
    np.maximum.at(DB, rnd_of_node_row, cntB)
    DA[(DA + DB) == 0] = 1

    # greedy grouping: consecutive rounds, group-uniform (da, db), bounded
    # gather-slice budget and PSUM batch width
    groups = []  # [i0, nr, da, db]
    i = 0
    while i < R:
        da, db = int(DA[i]), int(DB[i])
        nr = 1
        while i + nr < R and nr < GROUPR:
            nda = max(da, int(DA[i + nr]))
            ndb = max(db, int(DB[i + nr]))
            if (nr + 1) * (nda + ndb) > GBUDGET:
                break
            da, db = nda, ndb
            nr += 1
        groups.append([i, nr, da, db])
        i += nr

    # slot position within (dst, bank); negative gather indices first so the
    # final slot of each call is non-negative (HW drops trailing negatives)
    idxval = np.where(bank == 0, sr - baseA, sr - baseB)
    nonneg = (idxval >= 0).astype(np.int8)
    o2 = np.lexsort((nonneg, bank, dr))
    grp2 = dr[o2] * 2 + bank[o2]
    uniq2, first2 = np.unique(grp2, return_index=True)
    dpos = np.arange(E) - first2[np.searchsorted(uniq2, grp2)]
    d_of_edge = np.empty(E, dtype=np.int64)
    d_of_edge[o2] = dpos

    # guard: if the node at partition 127 of a group's LAST round fills its
    # final column with a negative idx, the call would end on a trailing
    # negative -> widen the group by a pad col (idx 0 -> dummy row)
    cnt_nonneg_A = np.bincount(dr[(bank == 0) & (idxval >= 0)], minlength=NTOT)
    cnt_nonneg_B = np.bincount(dr[(bank == 1) & (idxval >= 0)], minlength=NTOT)
    p_of_row = np.empty(NTOT, dtype=np.int64)
    p_of_row[row_of_listpos] = p
    is127 = p_of_row == 127
    for g in groups:
        i0, nr, da, db = g
        selr = is127 & (rnd_of_node_row == i0 + nr - 1)
        if np.any(selr & (cntA == da) & (cntA > 0) & (cnt_nonneg_A == 0)):
            g[2] = da + 1
        if np.any(selr & (cntB == db) & (cntB > 0) & (cnt_nonneg_B == 0)):
            g[3] = db + 1

    gbaseA = [0]
    gbaseB = [0]
    for (i0, nr, da, db) in groups:
        gbaseA.append(gbaseA[-1] + nr * da)
        gbaseB.append(gbaseB[-1] + nr * db)
    SA, SB = int(gbaseA[-1]), int(gbaseB[-1])

    rbaseA = np.zeros(R, dtype=np.int64)
    rbaseB = np.zeros(R, dtype=np.int64)
    for gi, (i0, nr, da, db) in enumerate(groups):
        for r in range(nr):
            rbaseA[i0 + r] = gbaseA[gi] + r * da
            rbaseB[i0 + r] = gbaseB[gi] + r * db

    e_core = dr // NPC
    e_rnd = (dr % NPC) // 128
    e_p = dr % 128
    idxA = np.zeros((C, SA * 128), dtype=np.int32)
    idxB = np.zeros((C, SB * 128), dtype=np.int32)
    isA = bank == 0
    slotA = (rbaseA[e_rnd[isA]] + d_of_edge[isA]) * 128 + e_p[isA]
    idxA[e_core[isA], slotA] = sr[isA] - baseA
    isB = ~isA
    slotB = (rbaseB[e_rnd[isB]] + d_of_edge[isB]) * 128 + e_p[isB]
    idxB[e_core[isB], slotB] = sr[isB] - baseB
    assert idxA.min() >= -32768 and idxA.max() <= 32766
    assert idxB.min() >= -32768 and idxB.max() <= 32766
    assert not np.any(idxA == -1) and not np.any(idxB == -1)

    def wrap(a):  # [C, S*128] -> [C, 128, S*8] int16 (16-wrap, replicated x8)
        Cn, tot = a.shape
        w = a.reshape(Cn, tot // 16, 16).transpose(0, 2, 1)
        return np.ascontiguousarray(np.tile(w, (1, 8, 1))).astype(np.int16)

    xf = np.asarray(x, dtype=np.float32)
    xT = np.zeros((F, NTOT), dtype=np.float16)
    xT[:, row_of_listpos[real]] = xf[node_at_listpos[real]].T.astype(np.float16)

    return dict(
        idxA=wrap(idxA), idxB=wrap(idxB), xT=xT,
        groups=[tuple(g) for g in groups], SA=SA, SB=SB,
        row_of_node=row_of_node,
    )


def _build(cfg, groups, SA, SB):
    import sys
    if "/opt/trn_rl_repo" not in sys.path:
        sys.path.insert(0, "/opt/trn_rl_repo")
    import concourse.mybir as mybir
    import concourse.tile as tile
    from concourse import bacc
    from concourse.masks import make_identity

    f32 = mybir.dt.float32
    f16 = mybir.dt.float16
    R, NPC, NTOT = cfg["R"], cfg["NPC"], cfg["NTOT"]
    baseA, baseB = cfg["baseA"], cfg["baseB"]
    NB = NTOT // 128          # 784 chunks
    BCH = 7                   # chunks per PSUM batch
    GMAXA = max(g[1] * g[2] for g in groups)
    GMAXB = max(g[1] * g[3] for g in groups)
    NSP = 128 * SPLIT_R       # h1 AllGather split point (cols per core)

    nc = bacc.Bacc("TRN2", target_bir_lowering=False, debug=False, num_devices=8)
    xT_t = nc.dram_tensor("xT", [F, NTOT], f16, kind="ExternalInput")
    xTo_t = nc.dram_tensor("xTo", [F, NPC], f16, kind="ExternalInput")
    iA_t = nc.dram_tensor("idxA", [128, SA * 8], mybir.dt.int16, kind="ExternalInput")
    iB_t = nc.dram_tensor("idxB", [128, SB * 8], mybir.dt.int16, kind="ExternalInput")
    aug1_t = nc.dram_tensor("aug1", [F, HD + 2], f16, kind="ExternalInput")
    aug2_t = nc.dram_tensor("aug2", [HD, HD + 2], f16, kind="ExternalInput")
    bv_t = nc.dram_tensor("bvec", [2, HD], f32, kind="ExternalInput")
    out_t = nc.dram_tensor("out", [NPC, HD], f32, kind="ExternalOutput")

    table1 = nc.dram_tensor("table1", [NTOT, ROWW], f16, kind="Internal")
    table2 = nc.dram_tensor("table2", [NTOT, ROWW], f16, kind="Internal")
    h1sh = nc.dram_tensor("h1sh", [HD, NPC], f16, kind="Internal")
    h1all = nc.dram_tensor("h1all", [C * HD, NPC], f16, kind="Internal",
                           addr_space="Shared")
    RG = [[0, 1, 2, 3, 4, 5, 6, 7]]

    with tile.TileContext(nc) as tc:
        with tc.tile_pool(name="const", bufs=1) as cp, \
             tc.tile_pool(name="bsrc", bufs=2) as bp, \
             tc.tile_pool(name="bstg", bufs=2) as sp, \
             tc.tile_pool(name="gpool", bufs=2) as gp, \
             tc.tile_pool(name="mpool", bufs=2) as mp, \
             tc.tile_pool(name="psB", bufs=2, space="PSUM") as psB, \
             tc.tile_pool(name="psW", bufs=1, space="PSUM") as psW, \
             tc.tile_pool(name="psT", bufs=2, space="PSUM") as psT:

            ident = cp.tile([128, 128], f16)
            make_identity(nc, ident[:])

            aug1 = cp.tile([F, HD + 2], f16, tag="aug1")
            nc.sync.dma_start(out=aug1[:], in_=aug1_t.ap()[:, :])
            aug2 = cp.tile([HD, HD + 2], f16, tag="aug2")
            nc.sync.dma_start(out=aug2[:], in_=aug2_t.ap()[:, :])
            augs = [aug1, aug2]

            bb = []
            for l in range(2):
                t = cp.tile([128, HD], f32, tag=f"b{l}")
                nc.sync.dma_start(out=t[:1, :], in_=bv_t.ap()[l:l + 1, :])
                nc.gpsimd.partition_broadcast(t[:], t[:1, :])
                bb.append(t)

            shft = cp.tile([128, 1], f32, tag="shft")
            nc.vector.memset(shft[:], EXP_SHIFT)

            dumrow = cp.tile([1, ROWW], f16)
            nc.vector.memset(dumrow[:], 0.0)
            nc.vector.memset(dumrow[:, HD:HD + 1], -60.0)

            iA_sb = cp.tile([128, SA * 8], mybir.dt.int16)
            nc.sync.dma_start(out=iA_sb[:], in_=iA_t.ap()[:, :])
            iB_sb = cp.tile([128, SB * 8], mybir.dt.int16)
            nc.sync.dma_start(out=iB_sb[:], in_=iB_t.ap()[:, :])

            adst0 = cp.tile([128, R], f32, tag="adst0")
            adst1 = cp.tile([128, R], f32, tag="adst1")
            adst = [adst0, adst1]
            h1T = cp.tile([HD, NPC], f16, tag="h1T")

            def build_block(layer, table, src_t, src_row0, src_col0, c0, nch):
                """Build table rows for chunks [c0, c0+nch) from src_t's
                columns starting at src_col0.  nch is a multiple of BCH."""
                aug = augs[layer]
                K = F if layer == 0 else HD
                srct = bp.tile([128, 4 * BCH * 128], f16, tag="bsrc")
                nc.sync.dma_start(
                    out=srct[:K, 0:nch * 128],
                    in_=src_t.ap()[src_row0:src_row0 + K,
                                   src_col0:src_col0 + nch * 128])
                st = sp.tile([128, 4 * BCH, AF], f16, tag="bst")
                for q in range(nch // BCH):
                    ps = psB.tile([128, BCH * AF], f32, tag="psb")
                    for j in range(BCH):
                        jj = q * BCH + j
                        nc.tensor.matmul(
                            out=ps[:, AF * j:AF * (j + 1)],
                            lhsT=srct[:K, 128 * jj:128 * (jj + 1)],
                            rhs=aug[:K, 0:AF], start=True, stop=True)
                    nc.scalar.copy(
                        out=st[:, q * BCH:(q + 1) * BCH, :],
                        in_=ps[:].rearrange("p (j a) -> p j a", j=BCH))
                nc.sync.dma_start(
                    out=table.ap()[128 * c0:128 * (c0 + nch),
                                   0:AF].rearrange("(j p) a -> p j a", p=128),
                    in_=st[:, 0:nch, :])

            def build_table1():
                for blk in range(NB // (4 * BCH)):
                    build_block(0, table1, xT_t, 0, 4 * BCH * 128 * blk,
                                4 * BCH * blk, 4 * BCH)
                nc.gpsimd.dma_start(out=table1.ap()[baseA:baseA + 1, :],
                                    in_=dumrow[:])
                nc.gpsimd.dma_start(out=table1.ap()[baseB:baseB + 1, :],
                                    in_=dumrow[:])

            def build_table2():
                # rank-striped source; 14-chunk blocks
                for rk in range(C):
                    for b in range(R // (2 * BCH)):
                        r0 = 2 * BCH * b
                        build_block(1, table2, h1all, HD * rk,
                                    128 * r0, rk * R + r0, 2 * BCH)
                nc.gpsimd.dma_start(out=table2.ap()[baseA:baseA + 1, :],
                                    in_=dumrow[:])
                nc.gpsimd.dma_start(out=table2.ap()[baseB:baseB + 1, :],
                                    in_=dumrow[:])

            def adst_own_l1():
                for blk in range(NPC // 128 // BCH):
                    srct = bp.tile([F, BCH * 128], f16, tag="osrc")
                    nc.sync.dma_start(
                        out=srct[:],
                        in_=xTo_t.ap()[:, BCH * 128 * blk:BCH * 128 * (blk + 1)])
                    ps = psW.tile([128, BCH], f32, tag="pso")
                    for j in range(BCH):
                        nc.tensor.matmul(
                            out=ps[:, j:j + 1],
                            lhsT=srct[:, 128 * j:128 * (j + 1)],
                            rhs=aug1[:, HD + 1:HD + 2], start=True, stop=True)
                    nc.vector.tensor_copy(
                        out=adst[0][:, BCH * blk:BCH * (blk + 1)], in_=ps[:])

            gbA = [0]
            gbB = [0]
            for (i0, nr, da, db) in groups:
                gbA.append(gbA[-1] + nr * da)
                gbB.append(gbB[-1] + nr * db)

            def phase_gather(layer, table):
                final = layer == 1
                for gi, (i0, nr, da, db) in enumerate(groups):
                    GA = gp.tile([128, GMAXA, ROWW], f16, tag="GA")
                    GB = gp.tile([128, GMAXB, ROWW], f16, tag="GB")
                    if da:
                        nc.gpsimd.dma_gather(
                            out_ap=GA[:, 0:nr * da, :], in_ap=table.ap()[baseA:, :],
                            idxs_ap=iA_sb[:, gbA[gi] * 8:gbA[gi + 1] * 8],
                            num_idxs=128 * nr * da, num_idxs_reg=128 * nr * da,
                            elem_size=ROWW, single_packet=False)
                    if db:
                        nc.gpsimd.dma_gather(
                            out_ap=GB[:, 0:nr * db, :], in_ap=table.ap()[baseB:, :],
                            idxs_ap=iB_sb[:, gbB[gi] * 8:gbB[gi + 1] * 8],
                            num_idxs=128 * nr * db, num_idxs_reg=128 * nr * db,
                            elem_size=ROWW, single_packet=False)
                    po = psB.tile([128, GROUPR * AF], f32, tag="po")
                    adcol3 = adst[layer][:, i0:i0 + nr].rearrange(
                        "p (r one) -> p r one", one=1)
                    MM = [None, None]
                    for qi, (G, GMAXQ, dq, tagq) in enumerate(
                            ((GA, GMAXA, da, "a"), (GB, GMAXB, db, "b"))):
                        if dq == 0:
                            continue
                        S = nr * dq
                        asrc3 = G[:, 0:S, HD:HD + 1].rearrange(
                            "p (r d) one -> p r (d one)", r=nr)
                        z = mp.tile([128, GMAXQ], f32, tag="z" + tagq)
                        nc.vector.tensor_tensor(
                            out=z[:, 0:S].rearrange("p (r d) -> p r d", r=nr),
                            in0=asrc3,
                            in1=adcol3.to_broadcast([128, nr, dq]),
                            op=mybir.AluOpType.add)
                        u = mp.tile([128, GMAXQ], f32, tag="u" + tagq)
                        nc.vector.tensor_scalar(
                            out=u[:, 0:S], in0=z[:, 0:S],
                            scalar1=NEG_SLOPE, scalar2=EXP_SHIFT,
                            op0=mybir.AluOpType.mult, op1=mybir.AluOpType.add)
                        e1 = mp.tile([128, GMAXQ], f32, tag="e1" + tagq)
                        nc.scalar.activation(
                            out=e1[:, 0:S], in_=z[:, 0:S],
                            func=mybir.ActivationFunctionType.Exp, bias=shft[:])
                        e2 = mp.tile([128, GMAXQ], f32, tag="e2" + tagq)
                        nc.scalar.activation(
                            out=e2[:, 0:S], in_=u[:, 0:S],
                            func=mybir.ActivationFunctionType.Exp)
                        # tt duplicated in pairs: a stride-1 len-2 last dim
                        # keeps every operand of the big multiply 2x-eligible
                        # on the DVE (a stride-0 broadcast would force 1x)
                        ttd = mp.tile([128, GMAXQ, 2], f16, tag="ttd" + tagq)
                        nc.vector.tensor_tensor(
                            out=ttd[:, 0:S, :],
                            in0=e1[:, 0:S].rearrange(
                                "p (s one) -> p s one", one=1).to_broadcast(
                                [128, S, 2]),
                            in1=e2[:, 0:S].rearrange(
                                "p (s one) -> p s one", one=1).to_broadcast(
                                [128, S, 2]),
                            op=mybir.AluOpType.max)
                        M = mp.tile([128, GMAXQ, AF], f16, tag="M" + tagq)
                        nc.vector.tensor_copy(
                            out=M[:, 0:S, HD:HD + 1], in_=ttd[:, 0:S, 0:1])
                        nc.vector.tensor_tensor(
                            out=M[:, 0:S, 0:HD].rearrange(
                                "p s (f two) -> p s f two", two=2),
                            in0=G[:, 0:S, 0:HD].rearrange(
                                "p s (f two) -> p s f two", two=2),
                            in1=ttd[:, 0:S, :].rearrange(
                                "p s (one two) -> p s one two", two=2
                                ).to_broadcast([128, S, HD // 2, 2]),
                            op=mybir.AluOpType.mult)
                        MM[qi] = M
                    lastq = 1 if MM[1] is not None else 0
                    for r in range(nr):
                        started = False
                        for qi, dq in ((0, da), (1, db)):
                            M = MM[qi]
                            if M is None:
                                continue
                            for d in range(dq):
                                nc.tensor.matmul(
                                    out=po[:, AF * r:AF * (r + 1)],
                                    lhsT=ident[:], rhs=M[:, r * dq + d, :],
                                    start=not started,
                                    stop=(qi == lastq and d == dq - 1))
                                started = True
                    pov = po[:, 0:nr * AF].rearrange("p (r a) -> p r a", r=nr)
                    den = mp.tile([128, GROUPR, 1], f32, tag="den")
                    nc.vector.tensor_scalar_max(
                        out=den[:, 0:nr, :], in0=pov[:, :, HD:HD + 1],
                        scalar1=1e-30)
                    rd = mp.tile([128, GROUPR, 1], f32, tag="rd")
                    nc.vector.reciprocal(out=rd[:, 0:nr, :], in_=den[:, 0:nr, :])
                    h = mp.tile([128, GROUPR, HD], f32, tag="hf")
                    nc.vector.tensor_tensor(
                        out=h[:, 0:nr, :], in0=pov[:, :, 0:HD],
                        in1=rd[:, 0:nr, :].to_broadcast([128, nr, HD]),
                        op=mybir.AluOpType.mult)
                    nc.vector.tensor_tensor(
                        out=h[:, 0:nr, :], in0=h[:, 0:nr, :],
                        in1=bb[layer][:].rearrange(
                            "p (one a) -> p one a", one=1).to_broadcast(
                            [128, nr, HD]),
                        op=mybir.AluOpType.add)
                    if final:
                        nc.sync.dma_start(
                            out=out_t.ap()[128 * i0:128 * (i0 + nr), :].rearrange(
                                "(r p) a -> p r a", p=128),
                            in_=h[:, 0:nr, :])
                    else:
                        hf = mp.tile([128, GROUPR, HD], f16, tag="hfp")
                        nc.scalar.activation(
                            out=hf[:, 0:nr, :], in_=h[:, 0:nr, :],
                            func=mybir.ActivationFunctionType.Relu)
                        pst = psT.tile([HD, GROUPR * 128], f16, tag="pst")
                        for r in range(nr):
                            nc.tensor.transpose(
                                out=pst[:, 128 * r:128 * (r + 1)],
                                in_=hf[:, r, :], identity=ident[:])
                        nc.vector.tensor_copy(
                            out=h1T[:, 128 * i0:128 * (i0 + nr)],
                            in_=pst[:, 0:128 * nr])
                        psd = psW.tile([128, GROUPR], f32, tag="psd")
                        for r in range(nr):
                            nc.tensor.matmul(
                                out=psd[:, r:r + 1],
                                lhsT=h1T[:, 128 * (i0 + r):128 * (i0 + r + 1)],
                                rhs=aug2[:, HD + 1:HD + 2], start=True, stop=True)
                        nc.vector.tensor_copy(
                            out=adst[1][:, i0:i0 + nr], in_=psd[:, 0:nr])
            build_table1()
            adst_own_l1()
            phase_gather(0, table1)
            nc.sync.dma_start(out=h1sh.ap()[:, :], in_=h1T[:])
            nc.gpsimd.collective_compute(
                "AllGather", mybir.AluOpType.bypass, RG,
                ins=[h1sh.ap()[:, :]], outs=[h1all.ap()[:, :]])
            build_table2()
            phase_gather(1, table2)

    nc.compile()
    return nc


def _prep_all(inputs):
    x = np.asarray(inputs["x"])
    cfg = _make_cfg(x.shape[0])
    prep = _host_prep(x, inputs["edge_index"], cfg)
    key = (cfg["N"], prep["SA"], prep["SB"], tuple(prep["groups"]))
    if key not in _CACHE:
        _CACHE[key] = _build(cfg, prep["groups"], prep["SA"], prep["SB"])
    nc = _CACHE[key]

    W1 = np.asarray(inputs["W1"], dtype=np.float32)
    W2 = np.asarray(inputs["W2"], dtype=np.float32)
    aug1 = np.concatenate(
        [W1, (W1 @ np.asarray(inputs["a1_src"], np.float32))[:, None],
         (W1 @ np.asarray(inputs["a1_dst"], np.float32))[:, None]], axis=1
    ).astype(np.float16)
    aug2 = np.concatenate(
        [W2, (W2 @ np.asarray(inputs["a2_src"], np.float32))[:, None],
         (W2 @ np.asarray(inputs["a2_dst"], np.float32))[:, None]], axis=1
    ).astype(np.float16)
    bvec = np.stack([np.asarray(inputs["b1"]), np.asarray(inputs["b2"])]
                    ).astype(np.float32)
    NPC = cfg["NPC"]
    in_maps = []
    for k in range(C):
        in_maps.append({
            "xT": prep["xT"],
            "xTo": np.ascontiguousarray(prep["xT"][:, NPC * k:NPC * (k + 1)]),
            "idxA": prep["idxA"][k], "idxB": prep["idxB"][k],
            "aug1": aug1, "aug2": aug2, "bvec": bvec,
        })
    return nc, in_maps, prep, cfg


def prepped_run_args(inputs):
    nc, in_maps, prep, cfg = _prep_all(inputs)
    return nc, in_maps


def kernel(x, edge_index, W1, a1_src, a1_dst, b1, W2, a2_src, a2_dst, b2):
    import sys
    if "/opt/trn_rl_repo" not in sys.path:
        sys.path.insert(0, "/opt/trn_rl_repo")
    from concourse import bass_utils

    inputs = dict(x=x, edge_index=edge_index, W1=W1, a1_src=a1_src,
                  a1_dst=a1_dst, b1=b1, W2=W2, a2_src=a2_src, a2_dst=a2_dst,
                  b2=b2)
    nc, in_maps, prep, cfg = _prep_all(inputs)
    res = bass_utils.run_bass_kernel_spmd(nc, in_maps, core_ids=list(range(C)))
    shards = np.concatenate([res.results[k]["out"] for k in range(C)], axis=0)
    return shards[prep["row_of_node"]].astype(np.float32)
